# revision 5
# baseline (speedup 1.0000x reference)
# Sparsemax (entmax-2) attention kernel for Trainium2, 8 NeuronCores.
#
# Problem: q,k,v [2,16,2048,64] fp32; scores = (q @ k^T) / 8.0;
#          attn = sparsemax(scores) rowwise; out = attn @ v.
#
# Per core (batch*heads sharded 4 heads/core), per head:
#   - Q^T (scaled 1/temp) and K^T staged fp16 via PE transpose + ACT copy.
#   - S = Q K^T fp16 matmul into PSUM half-tiles [128,1024]; DVE InstMax
#     top-8 per half DIRECTLY from PSUM. DVE does ONLY the two max8 scans
#     per tile (the irreducible ~2.4us/tile top-k scan).
#   - Sparsemax tau per row without sort (halves variant): valid supports are
#     unions of per-half sorted prefixes; tau = max over the 9x9 prefix grid.
#     The whole chain (18-wide prefix scan, 81-wide grid, min-reduce) runs on
#     the otherwise-idle GpSimd engine (SBUF-only ops, tiny widths).
#   - A = relu(S + negtau) fused into the PSUM->SBUF evacuation on ACT
#     (per-partition bias): no separate fp16 S copy, no separate relu pass.
#     PSUM half-tiles live until the lagged relu (3-buf rotation = 6 banks,
#     + 2 banks AV accumulator = 8 banks exactly).
#   - A^T via DMA-transpose with tq-major ATb layout so each transpose's
#     writes are contiguous 4KB runs per partition (fast xbar path).
#   - AV with V STATIONARY: oT[d,q] += Vb[c]^T @ A^T[c] per k-chunk; output
#     kept transposed [D,S] in DRAM; host un-transposes on gather.
#   - Tile t's chain/relu/transpose are emitted one tile later (1-tile lag)
#     so every engine queue's waits are already satisfied when reached.
#
# kernel(**inputs) takes FULL inputs and returns the FULL output.

import numpy as np

B, H, S, D = 2, 16, 2048, 64
NCORES = 8
HPC = (B * H) // NCORES          # heads per core
QT = S // 128                    # 128-row query tiles per head
TEMP = 8.0
NGRID = 9
NCST = NGRID * NGRID

_cache = {}


def _consts():
    mn = np.arange(NGRID, dtype=np.float64)
    den = mn[:, None] + mn[None, :]
    R = np.where(den > 0, 1.0 / np.maximum(den, 1), 0.0)
    negR = -R
    negR[0, 0] = -1e30           # empty-set combo -> +1e30 after (Xg-1)*negR
    negR = np.broadcast_to(negR.reshape(1, NCST), (128, NCST))
    ident = np.eye(128)
    return np.concatenate([negR, ident], axis=1).astype(np.float32)


def build_program(hpc=None, reps=1):
    hpc = HPC if hpc is None else hpc
    key = (hpc, reps)
    if key in _cache:
        return _cache[key]

    import concourse.mybir as mybir
    import concourse.tile as tile
    from concourse import bacc

    f32 = mybir.dt.float32
    f16 = mybir.dt.float16
    AF = mybir.ActivationFunctionType
    ALU = mybir.AluOpType
    AX = mybir.AxisListType

    nc = bacc.Bacc(
        "TRN2",
        target_bir_lowering=False,
        debug=False,
        num_devices=NCORES,
    )

    q_d = nc.dram_tensor("q_s", [hpc, S, D], f32, kind="ExternalInput").ap()
    k_d = nc.dram_tensor("k_s", [hpc, S, D], f32, kind="ExternalInput").ap()
    v_d = nc.dram_tensor("v_s", [hpc, S, D], f32, kind="ExternalInput").ap()
    c_d = nc.dram_tensor("cst", [128, NCST + 128], f32, kind="ExternalInput").ap()
    o_d = nc.dram_tensor("o_s", [hpc, D, S], f32, kind="ExternalOutput").ap()

    from contextlib import ExitStack

    with tile.TileContext(nc) as tc, ExitStack() as ctx:
        cpool = ctx.enter_context(tc.tile_pool(name="consts", bufs=1))
        npool = ctx.enter_context(tc.tile_pool(name="nat", bufs=2))
        hpool = ctx.enter_context(tc.tile_pool(name="head", bufs=2))
        vpool = ctx.enter_context(tc.tile_pool(name="vb", bufs=2))
        apool = ctx.enter_context(tc.tile_pool(name="attn", bufs=3))
        tpool = ctx.enter_context(tc.tile_pool(name="atb", bufs=2))
        wpool = ctx.enter_context(tc.tile_pool(name="small", bufs=4))
        opool = ctx.enter_context(tc.tile_pool(name="outsb", bufs=2))
        sp = ctx.enter_context(tc.tile_pool(name="spsum", bufs=3, space="PSUM"))
        pout = ctx.enter_context(tc.tile_pool(name="pso", bufs=1, space="PSUM"))

        Ct = cpool.tile([128, NCST + 128], f32)
        nc.sync.dma_start(Ct, c_d)
        negR = Ct[:, 0:NCST]
        ident = Ct[:, NCST:NCST + 128]
        zeros18 = cpool.tile([128, 18], f32)
        nc.vector.memset(zeros18, 0.0)

        heads = {}     # h -> dict(QTh, KTh, Vb, ATb={g: tile})
        pending = {}   # tile tail state: gt -> (ps0, ps1, negtau, h, g, tq)

        def load_head(h):
            st = {}
            st["Qn"] = npool.tile([128, QT, D], f32, tag="qn", name="Qn")
            st["Kn"] = npool.tile([128, QT, D], f32, tag="kn", name="Kn")
            st["Vn"] = npool.tile([128, QT, D], f32, tag="vn", name="Vn")
            nc.sync.dma_start(st["Qn"], q_d[h].rearrange("(t p) d -> p t d", p=128))
            nc.sync.dma_start(st["Kn"], k_d[h].rearrange("(t p) d -> p t d", p=128))
            nc.sync.dma_start(st["Vn"], v_d[h].rearrange("(t p) d -> p t d", p=128))
            heads[h] = st

        def stage_head(h):
            st = heads[h]
            st["QTh"] = hpool.tile([64, S], f16, tag="qT", name="QTh")
            st["KTh"] = hpool.tile([64, S], f16, tag="kT", name="KTh")
            for src, dst, scale in ((st["Qn"], st["QTh"], 1.0 / TEMP),
                                    (st["Kn"], st["KTh"], 1.0)):
                for s2 in range(2):
                    pt = sp.tile([64, 1024], f32, tag="s")
                    for c in range(8):
                        nc.tensor.transpose(pt[:, c * 128:(c + 1) * 128],
                                            src[:, s2 * 8 + c, :], ident)
                    nc.scalar.activation(dst[:, s2 * 1024:(s2 + 1) * 1024],
                                         pt, AF.Copy, bias=0.0, scale=scale)
            st["Vb"] = vpool.tile([128, QT, D], f16, tag="vb", name="Vb")
            nc.gpsimd.tensor_copy(st["Vb"], st["Vn"])
            st["ATb"] = {}

        cands = {}

        def prep_cand(gt):
            # allocate + zero-pad next tile's cand one tile EARLY on Pool, so
            # the memset is never queued behind a waiting Pool op
            cand = wpool.tile([128, 18], f32, tag="cand", name="cand")
            nc.gpsimd.memset(cand[:, 0:10:9], 0.0)
            cands[gt] = cand

        def emit_head_tile(gt):
            h, t = divmod(gt, QT)
            g, tq = divmod(t, 8)
            st = heads[h]
            if tq == 0:
                # tq-major layout: [p, tq, c, qi] so each DMA transpose's
                # destination [:, tq, :, :] is contiguous per partition
                st["ATb"][g] = tpool.tile([128, 8, QT, 128], f16, tag="atb",
                                          name="ATb")

            if gt not in cands:
                prep_cand(gt)
            cand = cands.pop(gt)
            pss = []
            for hf in range(2):
                ps = sp.tile([128, 1024], f32, tag="s")
                for j in range(2):
                    nc.tensor.matmul(
                        ps[:, j * 512:(j + 1) * 512],
                        st["QTh"][:, t * 128:(t + 1) * 128],
                        st["KTh"][:, hf * 1024 + j * 512:
                                  hf * 1024 + (j + 1) * 512],
                        start=True, stop=True,
                    )
                nc.vector.max(cand[:, 1 + hf * 9:9 + hf * 9], ps)
                pss.append(ps)
            prep_cand(gt + 1)

            # tau chain: two tiny 9-wide prefix scans on DVE (initial=-1 on
            # the A half folds the "-1" so no per-partition correction is
            # needed); the 81-wide grid add + negR multiply run on the
            # otherwise-idle GpSimd engine (SBUF-only, walrus-supported ops)
            scr = wpool.tile([128, 18], f32, tag="scr")
            nc.vector.tensor_tensor_scan(scr[:, 0:9], cand[:, 0:9],
                                         zeros18[:, 0:9], -1.0,
                                         ALU.add, ALU.add)
            nc.vector.tensor_tensor_scan(scr[:, 9:18], cand[:, 9:18],
                                         zeros18[:, 0:9], 0.0,
                                         ALU.add, ALU.add)
            Xg = wpool.tile([128, NGRID, NGRID], f32, tag="xg")
            nc.gpsimd.tensor_tensor(
                out=Xg,
                in0=scr[:, 0:9].unsqueeze(2).broadcast_to([128, NGRID, NGRID]),
                in1=scr[:, 9:18].unsqueeze(1).broadcast_to([128, NGRID, NGRID]),
                op=ALU.add)
            Tg = wpool.tile([128, NCST], f32, tag="tg")
            nc.gpsimd.tensor_tensor(
                out=Tg, in0=Xg.rearrange("p a b -> p (a b)"), in1=negR,
                op=ALU.mult)
            negtau = wpool.tile([128, 1], f32, tag="nt")
            nc.vector.tensor_reduce(negtau, Tg, axis=AX.X, op=ALU.min)
            pending[gt] = (pss, negtau, h, g, tq)
            if gt - 1 in pending:
                emit_relu(gt - 1)

        def emit_relu(gt):
            # relu + transpose with a 1-tile lag: negtau is already computed
            # when the ACT queue reaches the relu; A = relu(S + negtau) is
            # fused into the PSUM->SBUF evacuation (frees the psum slots)
            pss, negtau, h, g, tq = pending.pop(gt)
            st = heads[h]
            Ab = apool.tile([128, S], f16, tag="ab")
            for hf in range(2):
                nc.scalar.activation(Ab[:, hf * 1024:(hf + 1) * 1024],
                                     pss[hf], AF.Relu, bias=negtau, scale=1.0)
            nc.sync.dma_start_transpose(st["ATb"][g][:, tq, :, :], Ab)

        av = {}  # in-flight AV accumulation state

        def pump_av(n):
            # emit next n k-chunks of the in-flight AV; finalize at 16
            if not av:
                return
            st = heads[av["h"]]
            ATbg = st["ATb"][av["g"]]
            for c in range(av["c"], min(av["c"] + n, QT)):
                for j in range(2):
                    nc.tensor.matmul(
                        av["po"][:, j * 512:(j + 1) * 512],
                        st["Vb"][:, c, :],
                        ATbg[:, j * 4:(j + 1) * 4, c, :],
                        start=(c == 0), stop=(c == QT - 1),
                    )
            av["c"] = min(av["c"] + n, QT)
            if av["c"] == QT:
                osb = opool.tile([64, 1024], f32, tag="osb")
                nc.scalar.activation(osb, av["po"], AF.Copy, bias=0.0, scale=1.0)
                nc.sync.dma_start(
                    o_d[av["h"], :, av["g"] * 1024:(av["g"] + 1) * 1024], osb)
                av.clear()

        def start_av(h, g):
            po = pout.tile([64, 1024], f32, tag="po", name=f"po_{h}_{g}")
            av.update(h=h, g=g, c=0, po=po)

        for _rep in range(reps):
            load_head(0)
            for h in range(hpc):
                if h + 1 < hpc:
                    load_head(h + 1)
                stage_head(h)
                for t in range(QT):
                    gt = h * QT + t
                    emit_head_tile(gt)
                    if t == 8:
                        start_av(h, 0)
                    if t == 0 and h > 0:
                        start_av(h - 1, 1)
                    pump_av(2)
            last = hpc * QT - 1
            if last in pending:
                emit_relu(last)
            cands.clear()
            start_av(hpc - 1, 1)
            pump_av(QT)
            heads.clear()
            pending.clear()

    nc.compile()
    _cache[key] = nc
    return nc


def shard_inputs(q, k, v):
    qf = np.ascontiguousarray(np.asarray(q).reshape(B * H, S, D), dtype=np.float32)
    kf = np.ascontiguousarray(np.asarray(k).reshape(B * H, S, D), dtype=np.float32)
    vf = np.ascontiguousarray(np.asarray(v).reshape(B * H, S, D), dtype=np.float32)
    cst = np.ascontiguousarray(_consts())
    in_maps = []
    for i in range(NCORES):
        sl = slice(i * HPC, (i + 1) * HPC)
        in_maps.append({
            "q_s": np.ascontiguousarray(qf[sl]),
            "k_s": np.ascontiguousarray(kf[sl]),
            "v_s": np.ascontiguousarray(vf[sl]),
            "cst": cst,
        })
    return in_maps


def unshard(results):
    # o_s per core: [HPC, D, S] transposed; -> [B, H, S, D]
    out = np.concatenate([r["o_s"] for r in results], axis=0)
    return np.ascontiguousarray(
        out.transpose(0, 2, 1).reshape(B, H, S, D).astype(np.float32))


def kernel(q, k, v):
    from concourse.bass_utils import run_bass_kernel_spmd

    nc = build_program()
    in_maps = shard_inputs(q, k, v)
    res = run_bass_kernel_spmd(nc, in_maps, core_ids=list(range(NCORES)))
    return unshard(res.results)


if __name__ == "__main__":
    rng = np.random.default_rng(0)
    q = rng.standard_normal((B, H, S, D), dtype=np.float32)
    k = rng.standard_normal((B, H, S, D), dtype=np.float32)
    v = rng.standard_normal((B, H, S, D), dtype=np.float32)
    o = kernel(q, k, v)
    print(o.shape, o.dtype)


# revision 9
# speedup vs baseline: 1.0275x; 1.0275x over previous
# Sparsemax (entmax-2) attention kernel for Trainium2, 8 NeuronCores.
#
# Problem: q,k,v [2,16,2048,64] fp32; scores = (q @ k^T) / 8.0;
#          attn = sparsemax(scores) rowwise; out = attn @ v.
#
# Per core (batch*heads sharded 4 heads/core), per head:
#   - Q^T (scaled 1/temp) and K^T staged fp16 via PE transpose + ACT copy.
#   - S = Q K^T fp16 matmul into PSUM half-tiles [128,1024]; DVE InstMax
#     top-8 per half DIRECTLY from PSUM. DVE does ONLY the two max8 scans
#     per tile (the irreducible ~2.4us/tile top-k scan).
#   - Sparsemax tau per row without sort (halves variant): valid supports are
#     unions of per-half sorted prefixes; tau = max over the 9x9 prefix grid.
#     The whole chain (18-wide prefix scan, 81-wide grid, min-reduce) runs on
#     the otherwise-idle GpSimd engine (SBUF-only ops, tiny widths).
#   - A = relu(S + negtau) fused into the PSUM->SBUF evacuation on ACT
#     (per-partition bias): no separate fp16 S copy, no separate relu pass.
#     PSUM half-tiles live until the lagged relu (3-buf rotation = 6 banks,
#     + 2 banks AV accumulator = 8 banks exactly).
#   - A^T via DMA-transpose with tq-major ATb layout so each transpose's
#     writes are contiguous 4KB runs per partition (fast xbar path).
#   - AV with V STATIONARY: oT[d,q] += Vb[c]^T @ A^T[c] per k-chunk; output
#     kept transposed [D,S] in DRAM; host un-transposes on gather.
#   - Tile t's chain/relu/transpose are emitted one tile later (1-tile lag)
#     so every engine queue's waits are already satisfied when reached.
#
# kernel(**inputs) takes FULL inputs and returns the FULL output.

import numpy as np

B, H, S, D = 2, 16, 2048, 64
NCORES = 8
HPC = (B * H) // NCORES          # heads per core
QT = S // 128                    # 128-row query tiles per head
TEMP = 8.0
NGRID = 9
NCST = NGRID * NGRID

_cache = {}


def _consts():
    mn = np.arange(NGRID, dtype=np.float64)
    den = mn[:, None] + mn[None, :]
    R = np.where(den > 0, 1.0 / np.maximum(den, 1), 0.0)
    negR = -R
    negR[0, 0] = -1e30           # empty-set combo -> +1e30 after (Xg-1)*negR
    negR = np.broadcast_to(negR.reshape(1, NCST), (128, NCST))
    ident = np.eye(128)
    return np.concatenate([negR, ident], axis=1).astype(np.float32)


def build_program(hpc=None, reps=1):
    hpc = HPC if hpc is None else hpc
    key = (hpc, reps)
    if key in _cache:
        return _cache[key]

    import concourse.mybir as mybir
    import concourse.tile as tile
    from concourse import bacc

    f32 = mybir.dt.float32
    f16 = mybir.dt.float16
    AF = mybir.ActivationFunctionType
    ALU = mybir.AluOpType
    AX = mybir.AxisListType

    nc = bacc.Bacc(
        "TRN2",
        target_bir_lowering=False,
        debug=False,
        num_devices=NCORES,
    )

    q_d = nc.dram_tensor("q_s", [hpc, S, D], f32, kind="ExternalInput").ap()
    k_d = nc.dram_tensor("k_s", [hpc, S, D], f32, kind="ExternalInput").ap()
    v_d = nc.dram_tensor("v_s", [hpc, S, D], f32, kind="ExternalInput").ap()
    c_d = nc.dram_tensor("cst", [128, NCST + 128], f32, kind="ExternalInput").ap()
    o_d = nc.dram_tensor("o_s", [hpc, D, S], f32, kind="ExternalOutput").ap()

    from contextlib import ExitStack

    with tile.TileContext(nc) as tc, ExitStack() as ctx:
        cpool = ctx.enter_context(tc.tile_pool(name="consts", bufs=1))
        npool = ctx.enter_context(tc.tile_pool(name="nat", bufs=2))
        hpool = ctx.enter_context(tc.tile_pool(name="head", bufs=2))
        vpool = ctx.enter_context(tc.tile_pool(name="vb", bufs=2))
        apool = ctx.enter_context(tc.tile_pool(name="attn", bufs=3))
        tpool = ctx.enter_context(tc.tile_pool(name="atb", bufs=2))
        wpool = ctx.enter_context(tc.tile_pool(name="small", bufs=4))
        opool = ctx.enter_context(tc.tile_pool(name="outsb", bufs=2))
        sp = ctx.enter_context(tc.tile_pool(name="spsum", bufs=3, space="PSUM"))
        pout = ctx.enter_context(tc.tile_pool(name="pso", bufs=1, space="PSUM"))

        Ct = cpool.tile([128, NCST + 128], f32)
        nc.sync.dma_start(Ct, c_d)
        negR = Ct[:, 0:NCST]
        ident = Ct[:, NCST:NCST + 128]
        zeros18 = cpool.tile([128, 18], f32)
        nc.vector.memset(zeros18, 0.0)

        heads = {}     # h -> dict(QTh, KTh, Vb, ATb={g: tile})
        pending = {}   # tile tail state: gt -> (ps0, ps1, negtau, h, g, tq)

        def load_head(h):
            st = {}
            st["Qn"] = npool.tile([128, QT, D], f32, tag="qn", name="Qn")
            st["Kn"] = npool.tile([128, QT, D], f32, tag="kn", name="Kn")
            st["Vn"] = npool.tile([128, QT, D], f32, tag="vn", name="Vn")
            nc.sync.dma_start(st["Qn"], q_d[h].rearrange("(t p) d -> p t d", p=128))
            nc.sync.dma_start(st["Kn"], k_d[h].rearrange("(t p) d -> p t d", p=128))
            nc.sync.dma_start(st["Vn"], v_d[h].rearrange("(t p) d -> p t d", p=128))
            heads[h] = st

        def stage_head(h):
            st = heads[h]
            st["QTh"] = hpool.tile([64, S], f16, tag="qT", name="QTh")
            st["KTh"] = hpool.tile([64, S], f16, tag="kT", name="KTh")
            for src, dst, scale in ((st["Qn"], st["QTh"], 1.0 / TEMP),
                                    (st["Kn"], st["KTh"], 1.0)):
                for s2 in range(2):
                    pt = sp.tile([64, 1024], f32, tag="s")
                    for c in range(8):
                        nc.tensor.transpose(pt[:, c * 128:(c + 1) * 128],
                                            src[:, s2 * 8 + c, :], ident)
                    nc.scalar.activation(dst[:, s2 * 1024:(s2 + 1) * 1024],
                                         pt, AF.Copy, bias=0.0, scale=scale)
            st["Vb"] = vpool.tile([128, QT, D], f16, tag="vb", name="Vb")
            nc.gpsimd.tensor_copy(st["Vb"], st["Vn"])
            st["ATb"] = {}

        cands = {}

        def prep_cand(gt):
            # allocate + zero-pad next tile's cand one tile EARLY on Pool, so
            # the memset is never queued behind a waiting Pool op
            cand = wpool.tile([128, 18], f32, tag="cand", name="cand")
            nc.gpsimd.memset(cand[:, 0:10:9], 0.0)
            cands[gt] = cand

        def emit_head_tile(gt):
            h, t = divmod(gt, QT)
            g, tq = divmod(t, 8)
            st = heads[h]
            if tq == 0:
                # tq-major layout: [p, tq, c, qi] so each DMA transpose's
                # destination [:, tq, :, :] is contiguous per partition
                st["ATb"][g] = tpool.tile([128, 8, QT, 128], f16, tag="atb",
                                          name="ATb")

            if gt not in cands:
                prep_cand(gt)
            cand = cands.pop(gt)
            pss = []
            for hf in range(2):
                ps = sp.tile([128, 1024], f32, tag="s")
                for j in range(2):
                    nc.tensor.matmul(
                        ps[:, j * 512:(j + 1) * 512],
                        st["QTh"][:, t * 128:(t + 1) * 128],
                        st["KTh"][:, hf * 1024 + j * 512:
                                  hf * 1024 + (j + 1) * 512],
                        start=True, stop=True,
                    )
                nc.vector.max(cand[:, 1 + hf * 9:9 + hf * 9], ps)
                pss.append(ps)
            prep_cand(gt + 1)

            # tau chain: two tiny 9-wide prefix scans on DVE (initial=-1 on
            # the A half folds the "-1" so no per-partition correction is
            # needed); the 81-wide grid add + negR multiply run on the
            # otherwise-idle GpSimd engine (SBUF-only, walrus-supported ops)
            scr = wpool.tile([128, 18], f32, tag="scr")
            nc.vector.tensor_tensor_scan(scr[:, 0:9], cand[:, 0:9],
                                         zeros18[:, 0:9], -1.0,
                                         ALU.add, ALU.add)
            nc.vector.tensor_tensor_scan(scr[:, 9:18], cand[:, 9:18],
                                         zeros18[:, 0:9], 0.0,
                                         ALU.add, ALU.add)
            Xg = wpool.tile([128, NGRID, NGRID], f32, tag="xg")
            nc.gpsimd.tensor_tensor(
                out=Xg,
                in0=scr[:, 0:9].unsqueeze(2).broadcast_to([128, NGRID, NGRID]),
                in1=scr[:, 9:18].unsqueeze(1).broadcast_to([128, NGRID, NGRID]),
                op=ALU.add)
            Tg = wpool.tile([128, NCST], f32, tag="tg")
            nc.gpsimd.tensor_tensor(
                out=Tg, in0=Xg.rearrange("p a b -> p (a b)"), in1=negR,
                op=ALU.mult)
            pending[gt] = (pss, Tg, h, g, tq)
            if gt - 1 in pending:
                emit_relu(gt - 1)

        def emit_relu(gt):
            # tau finalization + relu + transpose with a 1-tile lag: Pool's
            # grid add is already done when the DVE queue reaches the fused
            # multiply+min-reduce, and negtau is ready when the ACT queue
            # reaches the relu; A = relu(S + negtau) is fused into the
            # PSUM->SBUF evacuation (frees the psum slots)
            pss, Tg, h, g, tq = pending.pop(gt)
            st = heads[h]
            negtau = wpool.tile([128, 1], f32, tag="nt")
            nc.vector.tensor_reduce(negtau, Tg, axis=AX.X, op=ALU.min)
            Ab = apool.tile([128, S], f16, tag="ab")
            for hf in range(2):
                nc.scalar.activation(Ab[:, hf * 1024:(hf + 1) * 1024],
                                     pss[hf], AF.Relu, bias=negtau, scale=1.0)
            nc.sync.dma_start_transpose(st["ATb"][g][:, tq, :, :], Ab)

        av = {}  # in-flight AV accumulation state

        def pump_av(n):
            # emit next n k-chunks of the in-flight AV; finalize at 16
            if not av:
                return
            st = heads[av["h"]]
            ATbg = st["ATb"][av["g"]]
            for c in range(av["c"], min(av["c"] + n, QT)):
                for j in range(2):
                    nc.tensor.matmul(
                        av["po"][:, j * 512:(j + 1) * 512],
                        st["Vb"][:, c, :],
                        ATbg[:, j * 4:(j + 1) * 4, c, :],
                        start=(c == 0), stop=(c == QT - 1),
                    )
            av["c"] = min(av["c"] + n, QT)
            if av["c"] == QT:
                osb = opool.tile([64, 1024], f32, tag="osb")
                nc.scalar.activation(osb, av["po"], AF.Copy, bias=0.0, scale=1.0)
                nc.sync.dma_start(
                    o_d[av["h"], :, av["g"] * 1024:(av["g"] + 1) * 1024], osb)
                av.clear()

        def start_av(h, g):
            po = pout.tile([64, 1024], f32, tag="po", name=f"po_{h}_{g}")
            av.update(h=h, g=g, c=0, po=po)

        for _rep in range(reps):
            load_head(0)
            for h in range(hpc):
                if h + 1 < hpc:
                    load_head(h + 1)
                stage_head(h)
                for t in range(QT):
                    gt = h * QT + t
                    emit_head_tile(gt)
                    if t == 8:
                        start_av(h, 0)
                    if t == 0 and h > 0:
                        start_av(h - 1, 1)
                    pump_av(2)
            last = hpc * QT - 1
            if last in pending:
                emit_relu(last)
            cands.clear()
            start_av(hpc - 1, 1)
            pump_av(QT)
            heads.clear()
            pending.clear()

    nc.compile()
    _cache[key] = nc
    return nc


def shard_inputs(q, k, v):
    qf = np.ascontiguousarray(np.asarray(q).reshape(B * H, S, D), dtype=np.float32)
    kf = np.ascontiguousarray(np.asarray(k).reshape(B * H, S, D), dtype=np.float32)
    vf = np.ascontiguousarray(np.asarray(v).reshape(B * H, S, D), dtype=np.float32)
    cst = np.ascontiguousarray(_consts())
    in_maps = []
    for i in range(NCORES):
        sl = slice(i * HPC, (i + 1) * HPC)
        in_maps.append({
            "q_s": np.ascontiguousarray(qf[sl]),
            "k_s": np.ascontiguousarray(kf[sl]),
            "v_s": np.ascontiguousarray(vf[sl]),
            "cst": cst,
        })
    return in_maps


def unshard(results):
    # o_s per core: [HPC, D, S] transposed; -> [B, H, S, D]
    out = np.concatenate([r["o_s"] for r in results], axis=0)
    return np.ascontiguousarray(
        out.transpose(0, 2, 1).reshape(B, H, S, D).astype(np.float32))


def kernel(q, k, v):
    from concourse.bass_utils import run_bass_kernel_spmd

    nc = build_program()
    in_maps = shard_inputs(q, k, v)
    res = run_bass_kernel_spmd(nc, in_maps, core_ids=list(range(NCORES)))
    return unshard(res.results)


if __name__ == "__main__":
    rng = np.random.default_rng(0)
    q = rng.standard_normal((B, H, S, D), dtype=np.float32)
    k = rng.standard_normal((B, H, S, D), dtype=np.float32)
    v = rng.standard_normal((B, H, S, D), dtype=np.float32)
    o = kernel(q, k, v)
    print(o.shape, o.dtype)


# revision 11
# speedup vs baseline: 1.0338x; 1.0062x over previous
# Sparsemax (entmax-2) attention kernel for Trainium2, 8 NeuronCores.
#
# Problem: q,k,v [2,16,2048,64] fp32; scores = (q @ k^T) / 8.0;
#          attn = sparsemax(scores) rowwise; out = attn @ v.
#
# Per core (batch*heads sharded 4 heads/core), per head:
#   - Q^T (scaled 1/temp) and K^T staged fp16 via PE transpose + ACT copy.
#   - S = Q K^T fp16 matmul into PSUM half-tiles [128,1024]; DVE InstMax
#     top-8 per half DIRECTLY from PSUM. DVE does ONLY the two max8 scans
#     per tile (the irreducible ~2.4us/tile top-k scan).
#   - Sparsemax tau per row without sort (halves variant): valid supports are
#     unions of per-half sorted prefixes; tau = max over the 9x9 prefix grid.
#     The whole chain (18-wide prefix scan, 81-wide grid, min-reduce) runs on
#     the otherwise-idle GpSimd engine (SBUF-only ops, tiny widths).
#   - A = relu(S + negtau) fused into the PSUM->SBUF evacuation on ACT
#     (per-partition bias): no separate fp16 S copy, no separate relu pass.
#     PSUM half-tiles live until the lagged relu (3-buf rotation = 6 banks,
#     + 2 banks AV accumulator = 8 banks exactly).
#   - A^T via DMA-transpose with tq-major ATb layout so each transpose's
#     writes are contiguous 4KB runs per partition (fast xbar path).
#   - AV with V STATIONARY: oT[d,q] += Vb[c]^T @ A^T[c] per k-chunk; output
#     kept transposed [D,S] in DRAM; host un-transposes on gather.
#   - Tile t's chain/relu/transpose are emitted one tile later (1-tile lag)
#     so every engine queue's waits are already satisfied when reached.
#
# kernel(**inputs) takes FULL inputs and returns the FULL output.

import numpy as np

B, H, S, D = 2, 16, 2048, 64
NCORES = 8
HPC = (B * H) // NCORES          # heads per core
QT = S // 128                    # 128-row query tiles per head
TEMP = 8.0
NGRID = 9
NCST = NGRID * NGRID

_cache = {}


def _consts():
    mn = np.arange(NGRID, dtype=np.float64)
    den = mn[:, None] + mn[None, :]
    R = np.where(den > 0, 1.0 / np.maximum(den, 1), 0.0)
    negR = -R
    negR[0, 0] = -1e30           # empty-set combo -> +1e30 after (Xg-1)*negR
    negR = np.broadcast_to(negR.reshape(1, NCST), (128, NCST))
    ident = np.eye(128)
    return np.concatenate([negR, ident], axis=1).astype(np.float32)


def build_program(hpc=None, reps=1):
    hpc = HPC if hpc is None else hpc
    key = (hpc, reps)
    if key in _cache:
        return _cache[key]

    import concourse.mybir as mybir
    import concourse.tile as tile
    from concourse import bacc

    f32 = mybir.dt.float32
    f16 = mybir.dt.float16
    AF = mybir.ActivationFunctionType
    ALU = mybir.AluOpType
    AX = mybir.AxisListType

    nc = bacc.Bacc(
        "TRN2",
        target_bir_lowering=False,
        debug=False,
        num_devices=NCORES,
    )

    q_d = nc.dram_tensor("q_s", [hpc, S, D], f32, kind="ExternalInput").ap()
    k_d = nc.dram_tensor("k_s", [hpc, S, D], f32, kind="ExternalInput").ap()
    v_d = nc.dram_tensor("v_s", [hpc, S, D], f32, kind="ExternalInput").ap()
    c_d = nc.dram_tensor("cst", [128, NCST + 128], f32, kind="ExternalInput").ap()
    o_d = nc.dram_tensor("o_s", [hpc, D, S], f32, kind="ExternalOutput").ap()

    from contextlib import ExitStack

    with tile.TileContext(nc) as tc, ExitStack() as ctx:
        cpool = ctx.enter_context(tc.tile_pool(name="consts", bufs=1))
        npool = ctx.enter_context(tc.tile_pool(name="nat", bufs=2))
        hpool = ctx.enter_context(tc.tile_pool(name="head", bufs=2))
        vpool = ctx.enter_context(tc.tile_pool(name="vb", bufs=2))
        apool = ctx.enter_context(tc.tile_pool(name="attn", bufs=3))
        tpool = ctx.enter_context(tc.tile_pool(name="atb", bufs=2))
        wpool = ctx.enter_context(tc.tile_pool(name="small", bufs=4))
        opool = ctx.enter_context(tc.tile_pool(name="outsb", bufs=2))
        sp = ctx.enter_context(tc.tile_pool(name="spsum", bufs=3, space="PSUM"))
        pout = ctx.enter_context(tc.tile_pool(name="pso", bufs=1, space="PSUM"))

        Ct = cpool.tile([128, NCST + 128], f32)
        nc.sync.dma_start(Ct, c_d)
        negR = Ct[:, 0:NCST]
        ident = Ct[:, NCST:NCST + 128]
        zeros18 = cpool.tile([128, 18], f32)
        nc.vector.memset(zeros18, 0.0)

        heads = {}     # h -> dict(QTh, KTh, Vb, ATb={g: tile})
        pending = {}   # tile tail state: gt -> (ps0, ps1, negtau, h, g, tq)

        def load_head(h):
            st = {}
            st["Qn"] = npool.tile([128, QT, D], f32, tag="qn", name="Qn")
            st["Kn"] = npool.tile([128, QT, D], f32, tag="kn", name="Kn")
            st["Vn"] = npool.tile([128, QT, D], f32, tag="vn", name="Vn")
            nc.sync.dma_start(st["Qn"], q_d[h].rearrange("(t p) d -> p t d", p=128))
            nc.sync.dma_start(st["Kn"], k_d[h].rearrange("(t p) d -> p t d", p=128))
            nc.sync.dma_start(st["Vn"], v_d[h].rearrange("(t p) d -> p t d", p=128))
            heads[h] = st

        def stage_part(h, part):
            # one quarter of the Q^T/K^T staging (emitted spread across the
            # previous head's tile loop so head boundaries don't stall)
            st = heads[h]
            if part == 0:
                st["QTh"] = hpool.tile([64, S], f16, tag="qT", name="QTh")
                st["KTh"] = hpool.tile([64, S], f16, tag="kT", name="KTh")
                st["ATb"] = {}
            src, dst, scale = ((st["Qn"], st["QTh"], 1.0 / TEMP),
                               (st["Kn"], st["KTh"], 1.0))[part // 2]
            s2 = part % 2
            pt = sp.tile([64, 1024], f32, tag="s")
            for c in range(8):
                nc.tensor.transpose(pt[:, c * 128:(c + 1) * 128],
                                    src[:, s2 * 8 + c, :], ident)
            nc.scalar.activation(dst[:, s2 * 1024:(s2 + 1) * 1024],
                                 pt, AF.Copy, bias=0.0, scale=scale)

        def stage_v(h):
            st = heads[h]
            st["Vb"] = vpool.tile([128, QT, D], f16, tag="vb", name="Vb")
            nc.gpsimd.tensor_copy(st["Vb"], st["Vn"])

        def stage_head(h):
            for part in range(4):
                stage_part(h, part)
            stage_v(h)

        cands = {}

        def prep_cand(gt):
            # allocate + zero-pad next tile's cand one tile EARLY on Pool, so
            # the memset is never queued behind a waiting Pool op
            cand = wpool.tile([128, 18], f32, tag="cand", name="cand")
            nc.gpsimd.memset(cand[:, 0:10:9], 0.0)
            cands[gt] = cand

        def emit_head_tile(gt):
            h, t = divmod(gt, QT)
            g, tq = divmod(t, 8)
            st = heads[h]
            if tq == 0:
                # tq-major layout: [p, tq, c, qi] so each DMA transpose's
                # destination [:, tq, :, :] is contiguous per partition
                st["ATb"][g] = tpool.tile([128, 8, QT, 128], f16, tag="atb",
                                          name="ATb")

            if gt not in cands:
                prep_cand(gt)
            cand = cands.pop(gt)
            pss = []
            for hf in range(2):
                ps = sp.tile([128, 1024], f32, tag="s")
                for j in range(2):
                    nc.tensor.matmul(
                        ps[:, j * 512:(j + 1) * 512],
                        st["QTh"][:, t * 128:(t + 1) * 128],
                        st["KTh"][:, hf * 1024 + j * 512:
                                  hf * 1024 + (j + 1) * 512],
                        start=True, stop=True,
                    )
                nc.vector.max(cand[:, 1 + hf * 9:9 + hf * 9], ps)
                pss.append(ps)
            prep_cand(gt + 1)

            # tau chain: two tiny 9-wide prefix scans on DVE (initial=-1 on
            # the A half folds the "-1" so no per-partition correction is
            # needed); the 81-wide grid add + negR multiply run on the
            # otherwise-idle GpSimd engine (SBUF-only, walrus-supported ops)
            scr = wpool.tile([128, 18], f32, tag="scr")
            nc.vector.tensor_tensor_scan(scr[:, 0:9], cand[:, 0:9],
                                         zeros18[:, 0:9], -1.0,
                                         ALU.add, ALU.add)
            nc.vector.tensor_tensor_scan(scr[:, 9:18], cand[:, 9:18],
                                         zeros18[:, 0:9], 0.0,
                                         ALU.add, ALU.add)
            Xg = wpool.tile([128, NGRID, NGRID], f32, tag="xg")
            nc.gpsimd.tensor_tensor(
                out=Xg,
                in0=scr[:, 0:9].unsqueeze(2).broadcast_to([128, NGRID, NGRID]),
                in1=scr[:, 9:18].unsqueeze(1).broadcast_to([128, NGRID, NGRID]),
                op=ALU.add)
            Tg = wpool.tile([128, NCST], f32, tag="tg")
            nc.gpsimd.tensor_tensor(
                out=Tg, in0=Xg.rearrange("p a b -> p (a b)"), in1=negR,
                op=ALU.mult)
            pending[gt] = (pss, Tg, h, g, tq)
            if gt - 1 in pending:
                emit_relu(gt - 1)

        def emit_relu(gt):
            # tau finalization + relu + transpose with a 1-tile lag: Pool's
            # grid add is already done when the DVE queue reaches the fused
            # multiply+min-reduce, and negtau is ready when the ACT queue
            # reaches the relu; A = relu(S + negtau) is fused into the
            # PSUM->SBUF evacuation (frees the psum slots)
            pss, Tg, h, g, tq = pending.pop(gt)
            st = heads[h]
            negtau = wpool.tile([128, 1], f32, tag="nt")
            nc.vector.tensor_reduce(negtau, Tg, axis=AX.X, op=ALU.min)
            Ab = apool.tile([128, S], f16, tag="ab")
            for hf in range(2):
                nc.scalar.activation(Ab[:, hf * 1024:(hf + 1) * 1024],
                                     pss[hf], AF.Relu, bias=negtau, scale=1.0)
            nc.sync.dma_start_transpose(st["ATb"][g][:, tq, :, :], Ab)

        av = {}  # in-flight AV accumulation state

        def pump_av(n):
            # emit next n k-chunks of the in-flight AV; finalize at 16
            if not av:
                return
            st = heads[av["h"]]
            ATbg = st["ATb"][av["g"]]
            for c in range(av["c"], min(av["c"] + n, QT)):
                for j in range(2):
                    nc.tensor.matmul(
                        av["po"][:, j * 512:(j + 1) * 512],
                        st["Vb"][:, c, :],
                        ATbg[:, j * 4:(j + 1) * 4, c, :],
                        start=(c == 0), stop=(c == QT - 1),
                    )
            av["c"] = min(av["c"] + n, QT)
            if av["c"] == QT:
                osb = opool.tile([64, 1024], f32, tag="osb")
                nc.scalar.activation(osb, av["po"], AF.Copy, bias=0.0, scale=1.0)
                nc.sync.dma_start(
                    o_d[av["h"], :, av["g"] * 1024:(av["g"] + 1) * 1024], osb)
                av.clear()

        def start_av(h, g):
            po = pout.tile([64, 1024], f32, tag="po", name=f"po_{h}_{g}")
            av.update(h=h, g=g, c=0, po=po)

        for _rep in range(reps):
            load_head(0)
            for h in range(hpc):
                if h == 0:
                    stage_head(0)
                for t in range(QT):
                    gt = h * QT + t
                    emit_head_tile(gt)
                    if t == 8:
                        start_av(h, 0)
                    if t == 0 and h > 0:
                        start_av(h - 1, 1)
                    if h + 1 < hpc:
                        # prefetch next head: loads at t=6, staging spread
                        # over t=9..15 so the boundary never stalls
                        if t == 6:
                            load_head(h + 1)
                        elif t in (9, 11, 13, 15):
                            stage_part(h + 1, (t - 9) // 2)
                        elif t == 10:
                            stage_v(h + 1)
                    pump_av(2)
            last = hpc * QT - 1
            if last in pending:
                emit_relu(last)
            cands.clear()
            start_av(hpc - 1, 1)
            pump_av(QT)
            heads.clear()
            pending.clear()

    nc.compile()
    _cache[key] = nc
    return nc


def shard_inputs(q, k, v):
    qf = np.ascontiguousarray(np.asarray(q).reshape(B * H, S, D), dtype=np.float32)
    kf = np.ascontiguousarray(np.asarray(k).reshape(B * H, S, D), dtype=np.float32)
    vf = np.ascontiguousarray(np.asarray(v).reshape(B * H, S, D), dtype=np.float32)
    cst = np.ascontiguousarray(_consts())
    in_maps = []
    for i in range(NCORES):
        sl = slice(i * HPC, (i + 1) * HPC)
        in_maps.append({
            "q_s": np.ascontiguousarray(qf[sl]),
            "k_s": np.ascontiguousarray(kf[sl]),
            "v_s": np.ascontiguousarray(vf[sl]),
            "cst": cst,
        })
    return in_maps


def unshard(results):
    # o_s per core: [HPC, D, S] transposed; -> [B, H, S, D]
    out = np.concatenate([r["o_s"] for r in results], axis=0)
    return np.ascontiguousarray(
        out.transpose(0, 2, 1).reshape(B, H, S, D).astype(np.float32))


def kernel(q, k, v):
    from concourse.bass_utils import run_bass_kernel_spmd

    nc = build_program()
    in_maps = shard_inputs(q, k, v)
    res = run_bass_kernel_spmd(nc, in_maps, core_ids=list(range(NCORES)))
    return unshard(res.results)


if __name__ == "__main__":
    rng = np.random.default_rng(0)
    q = rng.standard_normal((B, H, S, D), dtype=np.float32)
    k = rng.standard_normal((B, H, S, D), dtype=np.float32)
    v = rng.standard_normal((B, H, S, D), dtype=np.float32)
    o = kernel(q, k, v)
    print(o.shape, o.dtype)


# revision 12
# speedup vs baseline: 1.0377x; 1.0037x over previous
# Sparsemax (entmax-2) attention kernel for Trainium2, 8 NeuronCores.
#
# Problem: q,k,v [2,16,2048,64] fp32; scores = (q @ k^T) / 8.0;
#          attn = sparsemax(scores) rowwise; out = attn @ v.
#
# Per core (batch*heads sharded 4 heads/core), per head:
#   - Q^T (scaled 1/temp) and K^T staged fp16 via PE transpose + ACT copy.
#   - S = Q K^T fp16 matmul into PSUM half-tiles [128,1024]; DVE InstMax
#     top-8 per half DIRECTLY from PSUM. DVE does ONLY the two max8 scans
#     per tile (the irreducible ~2.4us/tile top-k scan).
#   - Sparsemax tau per row without sort (halves variant): valid supports are
#     unions of per-half sorted prefixes; tau = max over the 9x9 prefix grid.
#     The whole chain (18-wide prefix scan, 81-wide grid, min-reduce) runs on
#     the otherwise-idle GpSimd engine (SBUF-only ops, tiny widths).
#   - A = relu(S + negtau) fused into the PSUM->SBUF evacuation on ACT
#     (per-partition bias): no separate fp16 S copy, no separate relu pass.
#     PSUM half-tiles live until the lagged relu (3-buf rotation = 6 banks,
#     + 2 banks AV accumulator = 8 banks exactly).
#   - A^T via DMA-transpose with tq-major ATb layout so each transpose's
#     writes are contiguous 4KB runs per partition (fast xbar path).
#   - AV with V STATIONARY: oT[d,q] += Vb[c]^T @ A^T[c] per k-chunk; output
#     kept transposed [D,S] in DRAM; host un-transposes on gather.
#   - Tile t's chain/relu/transpose are emitted one tile later (1-tile lag)
#     so every engine queue's waits are already satisfied when reached.
#
# kernel(**inputs) takes FULL inputs and returns the FULL output.

import numpy as np

B, H, S, D = 2, 16, 2048, 64
NCORES = 8
HPC = (B * H) // NCORES          # heads per core
QT = S // 128                    # 128-row query tiles per head
TEMP = 8.0
NGRID = 9
NCST = NGRID * NGRID

_cache = {}


def _consts():
    mn = np.arange(NGRID, dtype=np.float64)
    den = mn[:, None] + mn[None, :]
    R = np.where(den > 0, 1.0 / np.maximum(den, 1), 0.0)
    negR = -R
    negR[0, 0] = -1e30           # empty-set combo -> +1e30 after (Xg-1)*negR
    negR = np.broadcast_to(negR.reshape(1, NCST), (128, NCST))
    ident = np.eye(128)
    return np.concatenate([negR, ident], axis=1).astype(np.float32)


def build_program(hpc=None, reps=1):
    hpc = HPC if hpc is None else hpc
    key = (hpc, reps)
    if key in _cache:
        return _cache[key]

    import concourse.mybir as mybir
    import concourse.tile as tile
    from concourse import bacc

    f32 = mybir.dt.float32
    f16 = mybir.dt.float16
    AF = mybir.ActivationFunctionType
    ALU = mybir.AluOpType
    AX = mybir.AxisListType

    nc = bacc.Bacc(
        "TRN2",
        target_bir_lowering=False,
        debug=False,
        num_devices=NCORES,
    )

    q_d = nc.dram_tensor("q_s", [hpc, S, D], f32, kind="ExternalInput").ap()
    k_d = nc.dram_tensor("k_s", [hpc, S, D], f32, kind="ExternalInput").ap()
    v_d = nc.dram_tensor("v_s", [hpc, S, D], f32, kind="ExternalInput").ap()
    c_d = nc.dram_tensor("cst", [128, NCST + 128], f32, kind="ExternalInput").ap()
    o_d = nc.dram_tensor("o_s", [hpc, D, S], f32, kind="ExternalOutput").ap()

    from contextlib import ExitStack

    with tile.TileContext(nc) as tc, ExitStack() as ctx:
        cpool = ctx.enter_context(tc.tile_pool(name="consts", bufs=1))
        npool = ctx.enter_context(tc.tile_pool(name="nat", bufs=2))
        hpool = ctx.enter_context(tc.tile_pool(name="head", bufs=2))
        vpool = ctx.enter_context(tc.tile_pool(name="vb", bufs=2))
        apool = ctx.enter_context(tc.tile_pool(name="attn", bufs=3))
        tpool = ctx.enter_context(tc.tile_pool(name="atb", bufs=2))
        wpool = ctx.enter_context(tc.tile_pool(name="small", bufs=4))
        opool = ctx.enter_context(tc.tile_pool(name="outsb", bufs=2))
        sp = ctx.enter_context(tc.tile_pool(name="spsum", bufs=3, space="PSUM"))
        pout = ctx.enter_context(tc.tile_pool(name="pso", bufs=1, space="PSUM"))

        Ct = cpool.tile([128, NCST + 128], f32)
        nc.sync.dma_start(Ct, c_d)
        negR = Ct[:, 0:NCST]
        ident = Ct[:, NCST:NCST + 128]
        zeros18 = cpool.tile([128, 18], f32)
        nc.vector.memset(zeros18, 0.0)

        heads = {}     # h -> dict(QTh, KTh, Vb, ATb={g: tile})
        pending = {}   # tile tail state: gt -> (ps0, ps1, negtau, h, g, tq)

        def load_head(h):
            st = {}
            st["Qn"] = npool.tile([128, QT, D], f32, tag="qn", name="Qn")
            st["Kn"] = npool.tile([128, QT, D], f32, tag="kn", name="Kn")
            st["Vn"] = npool.tile([128, QT, D], f32, tag="vn", name="Vn")
            nc.sync.dma_start(st["Qn"], q_d[h].rearrange("(t p) d -> p t d", p=128))
            nc.sync.dma_start(st["Kn"], k_d[h].rearrange("(t p) d -> p t d", p=128))
            nc.sync.dma_start(st["Vn"], v_d[h].rearrange("(t p) d -> p t d", p=128))
            heads[h] = st

        def stage_part(h, part):
            # one quarter of the Q^T/K^T staging (emitted spread across the
            # previous head's tile loop so head boundaries don't stall)
            st = heads[h]
            if part == 0:
                st["QTh"] = hpool.tile([64, S], f16, tag="qT", name="QTh")
                st["KTh"] = hpool.tile([64, S], f16, tag="kT", name="KTh")
                st["ATb"] = {}
            src, dst, scale = ((st["Qn"], st["QTh"], 1.0 / TEMP),
                               (st["Kn"], st["KTh"], 1.0))[part // 2]
            s2 = part % 2
            pt = sp.tile([64, 1024], f32, tag="s")
            for c in range(8):
                nc.tensor.transpose(pt[:, c * 128:(c + 1) * 128],
                                    src[:, s2 * 8 + c, :], ident)
            nc.scalar.activation(dst[:, s2 * 1024:(s2 + 1) * 1024],
                                 pt, AF.Copy, bias=0.0, scale=scale)

        def stage_v(h):
            st = heads[h]
            st["Vb"] = vpool.tile([128, QT, D], f16, tag="vb", name="Vb")
            nc.gpsimd.tensor_copy(st["Vb"], st["Vn"])

        def stage_head(h):
            for part in range(4):
                stage_part(h, part)
            stage_v(h)

        cands = {}

        def prep_cand(gt):
            # allocate + zero-pad next tile's cand one tile EARLY on Pool, so
            # the memset is never queued behind a waiting Pool op
            cand = wpool.tile([128, 18], f32, tag="cand", name="cand")
            nc.gpsimd.memset(cand[:, 0:10:9], 0.0)
            cands[gt] = cand

        def emit_head_tile(gt):
            h, t = divmod(gt, QT)
            g, tq = divmod(t, 8)
            st = heads[h]
            if tq == 0:
                # tq-major layout: [p, tq, c, qi] so each DMA transpose's
                # destination [:, tq, :, :] is contiguous per partition
                st["ATb"][g] = tpool.tile([128, 8, QT, 128], f16, tag="atb",
                                          name="ATb")

            if gt not in cands:
                prep_cand(gt)
            cand = cands.pop(gt)
            pss = []
            for hf in range(2):
                ps = sp.tile([128, 1024], f32, tag="s")
                for j in range(2):
                    nc.tensor.matmul(
                        ps[:, j * 512:(j + 1) * 512],
                        st["QTh"][:, t * 128:(t + 1) * 128],
                        st["KTh"][:, hf * 1024 + j * 512:
                                  hf * 1024 + (j + 1) * 512],
                        start=True, stop=True,
                    )
                nc.vector.max(cand[:, 1 + hf * 9:9 + hf * 9], ps)
                pss.append(ps)
            prep_cand(gt + 1)

            # tau chain: two tiny 9-wide prefix scans on DVE (initial=-1 on
            # the A half folds the "-1" so no per-partition correction is
            # needed); the 81-wide grid add + negR multiply run on the
            # otherwise-idle GpSimd engine (SBUF-only, walrus-supported ops)
            scr = wpool.tile([128, 18], f32, tag="scr")
            nc.vector.tensor_tensor_scan(scr[:, 0:9], cand[:, 0:9],
                                         zeros18[:, 0:9], -1.0,
                                         ALU.add, ALU.add)
            nc.vector.tensor_tensor_scan(scr[:, 9:18], cand[:, 9:18],
                                         zeros18[:, 0:9], 0.0,
                                         ALU.add, ALU.add)
            Xg = wpool.tile([128, NGRID, NGRID], f32, tag="xg")
            nc.gpsimd.tensor_tensor(
                out=Xg,
                in0=scr[:, 0:9].unsqueeze(2).broadcast_to([128, NGRID, NGRID]),
                in1=scr[:, 9:18].unsqueeze(1).broadcast_to([128, NGRID, NGRID]),
                op=ALU.add)
            Tg = wpool.tile([128, NCST], f32, tag="tg")
            nc.gpsimd.tensor_tensor(
                out=Tg, in0=Xg.rearrange("p a b -> p (a b)"), in1=negR,
                op=ALU.mult)
            pending[gt] = (pss, Tg, h, g, tq)
            if gt - 1 in pending:
                emit_relu(gt - 1)

        def emit_relu(gt):
            # tau finalization + relu + transpose with a 1-tile lag: Pool's
            # grid add is already done when the DVE queue reaches the fused
            # multiply+min-reduce, and negtau is ready when the ACT queue
            # reaches the relu; A = relu(S + negtau) is fused into the
            # PSUM->SBUF evacuation (frees the psum slots)
            pss, Tg, h, g, tq = pending.pop(gt)
            st = heads[h]
            negtau = wpool.tile([128, 1], f32, tag="nt")
            nc.vector.tensor_reduce(negtau, Tg, axis=AX.X, op=ALU.min)
            Ab = apool.tile([128, S], f16, tag="ab")
            for hf in range(2):
                nc.scalar.activation(Ab[:, hf * 1024:(hf + 1) * 1024],
                                     pss[hf], AF.Relu, bias=negtau, scale=1.0)
            nc.sync.dma_start_transpose(st["ATb"][g][:, tq, :, :], Ab)

        av = {}  # in-flight AV accumulation state

        def pump_av(n):
            # emit next n k-chunks of the in-flight AV; finalize at 16
            if not av:
                return
            st = heads[av["h"]]
            ATbg = st["ATb"][av["g"]]
            for c in range(av["c"], min(av["c"] + n, QT)):
                for j in range(2):
                    nc.tensor.matmul(
                        av["po"][:, j * 512:(j + 1) * 512],
                        st["Vb"][:, c, :],
                        ATbg[:, j * 4:(j + 1) * 4, c, :],
                        start=(c == 0), stop=(c == QT - 1),
                    )
            av["c"] = min(av["c"] + n, QT)
            if av["c"] == QT:
                osb = opool.tile([64, 1024], f32, tag="osb")
                nc.scalar.activation(osb, av["po"], AF.Copy, bias=0.0, scale=1.0)
                nc.sync.dma_start(
                    o_d[av["h"], :, av["g"] * 1024:(av["g"] + 1) * 1024], osb)
                av.clear()

        def start_av(h, g):
            po = pout.tile([64, 1024], f32, tag="po", name=f"po_{h}_{g}")
            av.update(h=h, g=g, c=0, po=po)

        for _rep in range(reps):
            load_head(0)
            for h in range(hpc):
                if h == 0:
                    stage_head(0)
                for t in range(QT):
                    gt = h * QT + t
                    emit_head_tile(gt)
                    # pump BEFORE any start and start groups 1-2 tiles after
                    # their last transpose is emitted: the first AV matmul
                    # then never head-of-line-blocks the PE queue waiting on
                    # a DMA transpose (3/tile still completes 16 chunks with
                    # 2 tiles to spare)
                    pump_av(3)
                    if t == 9:
                        start_av(h, 0)
                    if t == 1 and h > 0:
                        start_av(h - 1, 1)
                    if h + 1 < hpc:
                        # prefetch next head: loads at t=6, staging spread
                        # over t=9..15 so the boundary never stalls
                        if t == 6:
                            load_head(h + 1)
                        elif t in (9, 11, 13, 15):
                            stage_part(h + 1, (t - 9) // 2)
                        elif t == 10:
                            stage_v(h + 1)
            last = hpc * QT - 1
            if last in pending:
                emit_relu(last)
            cands.clear()
            start_av(hpc - 1, 1)
            pump_av(QT)
            heads.clear()
            pending.clear()

    nc.compile()
    _cache[key] = nc
    return nc


def shard_inputs(q, k, v):
    qf = np.ascontiguousarray(np.asarray(q).reshape(B * H, S, D), dtype=np.float32)
    kf = np.ascontiguousarray(np.asarray(k).reshape(B * H, S, D), dtype=np.float32)
    vf = np.ascontiguousarray(np.asarray(v).reshape(B * H, S, D), dtype=np.float32)
    cst = np.ascontiguousarray(_consts())
    in_maps = []
    for i in range(NCORES):
        sl = slice(i * HPC, (i + 1) * HPC)
        in_maps.append({
            "q_s": np.ascontiguousarray(qf[sl]),
            "k_s": np.ascontiguousarray(kf[sl]),
            "v_s": np.ascontiguousarray(vf[sl]),
            "cst": cst,
        })
    return in_maps


def unshard(results):
    # o_s per core: [HPC, D, S] transposed; -> [B, H, S, D]
    out = np.concatenate([r["o_s"] for r in results], axis=0)
    return np.ascontiguousarray(
        out.transpose(0, 2, 1).reshape(B, H, S, D).astype(np.float32))


def kernel(q, k, v):
    from concourse.bass_utils import run_bass_kernel_spmd

    nc = build_program()
    in_maps = shard_inputs(q, k, v)
    res = run_bass_kernel_spmd(nc, in_maps, core_ids=list(range(NCORES)))
    return unshard(res.results)


if __name__ == "__main__":
    rng = np.random.default_rng(0)
    q = rng.standard_normal((B, H, S, D), dtype=np.float32)
    k = rng.standard_normal((B, H, S, D), dtype=np.float32)
    v = rng.standard_normal((B, H, S, D), dtype=np.float32)
    o = kernel(q, k, v)
    print(o.shape, o.dtype)
